# revision 6
# baseline (speedup 1.0000x reference)
"""Multi-head attention (B=4, L=2048, D=512, H=8) on 8 Trainium2 cores.

Sharding: core c handles batch b = c//2, query rows [(c%2)*1024, +1024).

Key trick: the key-mask zeroes ~half the KV positions and is known on the
host, so K/V are COMPACTED on the host to the unmasked positions (padded
to a multiple of 128; pad columns get a -1e30 score bias so exp()=0).
This halves scores/exp/attnV work. Each core projects the compacted K/V
for its whole batch itself (cheap), so no collective is needed.

Schedule: the scalar engine (exp) is the critical resource (~72 exps of
[128,1024]); everything else is arranged to keep it 100% busy:
 - warmup matmuls + a dummy exp run during the DMA preamble (HAM warm,
   ACT tables resident before the first real op)
 - only Q/K dmodel-chunk 0 is projected up front; attention starts
   immediately after, and V projection + remaining Q/K chunks are
   interleaved into the tensor engine's slack inside the head loops
 - per chunk, scores matmuls run one chunk ahead of attnV

Device layouts (per core):
  xqT (512, 1024), xkT/xvT (512, KVCAP)  inputs, dmodel on partitions
  qT (128, 1024) x4 / kT (128, KVCAP) x4 projections kept transposed:
      head h lives in dmodel-chunk tile h//2 at partition offset 64*(h%2)
  V (128, 520) x KVC   V natural layout per kv chunk; head h at cols
      [65h, 65h+64), col 65h+64 = ones (softmax denominator)
  scores (128kv, 1024q) PSUM; exp+mask+scale fused into one ACT op
  xs (65, 1024) PSUM, row 64 = softmax denominator
"""
import numpy as np
import ml_dtypes

import concourse.bacc as bacc
import concourse.bass as bass
import concourse.mybir as mybir
import concourse.tile as tile
from concourse.bass_utils import run_bass_kernel_spmd

F32 = mybir.dt.float32
BF16 = mybir.dt.bfloat16
AF = mybir.ActivationFunctionType

B, L, D = 4, 2048, 512
H, DK = 8, 64
N_CORES = 8
LQ = L // 2            # query rows per core
P = 128
QT = LQ // P           # 8 query tiles of 128
MC = D // P            # 4 dmodel chunks
MASK_BIAS = np.float32(-1e30)

MM_DT = BF16
MM_NP = ml_dtypes.bfloat16

_cache = {}


def _plan(mask):
    """KV chunk count after host-side compaction (multiple-of-128 pad)."""
    counts = np.asarray(mask).astype(bool).sum(axis=1)
    kvc = int(np.ceil((counts.max() + 1e-9) / P))
    return max(kvc, 2)


def _build(kvc):
    kvcap = kvc * P
    nc = bacc.Bacc("TRN2", target_bir_lowering=False, debug=False,
                   num_devices=N_CORES)

    xqT_d = nc.dram_tensor("xqT", [D, LQ], MM_DT, kind="ExternalInput").ap()
    xkT_d = nc.dram_tensor("xkT", [D, kvcap], MM_DT, kind="ExternalInput").ap()
    xvT_d = nc.dram_tensor("xvT", [D, kvcap], MM_DT, kind="ExternalInput").ap()
    wq_d = nc.dram_tensor("wq", [D, D], MM_DT, kind="ExternalInput").ap()
    wk_d = nc.dram_tensor("wk", [D, D], MM_DT, kind="ExternalInput").ap()
    wv_d = nc.dram_tensor("wv", [D, D], MM_DT, kind="ExternalInput").ap()
    wo_d = nc.dram_tensor("wo", [D, D], MM_DT, kind="ExternalInput").ap()
    bq_d = nc.dram_tensor("bq", [P, MC], F32, kind="ExternalInput").ap()
    bk_d = nc.dram_tensor("bk", [P, MC], F32, kind="ExternalInput").ap()
    bv_d = nc.dram_tensor("bv", [1, D], MM_DT, kind="ExternalInput").ap()
    bo2_d = nc.dram_tensor("bo2", [1, 2 * D], F32, kind="ExternalInput").ap()
    mb_d = nc.dram_tensor("mb", [P, kvc], F32, kind="ExternalInput").ap()
    out_d = nc.dram_tensor("out", [LQ, D], F32, kind="ExternalOutput").ap()

    # 512-column blocks of the compacted KV extent
    kblk = []
    off = 0
    while off < kvcap:
        blk = min(512, kvcap - off)
        kblk.append((off, blk))
        off += blk

    with tile.TileContext(nc) as tc:
        with tc.tile_pool(name="const", bufs=1) as cpool, \
             tc.tile_pool(name="xin", bufs=1) as xpool, \
             tc.tile_pool(name="proj", bufs=1) as prpool, \
             tc.tile_pool(name="attn", bufs=6) as apool, \
             tc.tile_pool(name="norm", bufs=2) as npool, \
             tc.tile_pool(name="outp", bufs=2) as opool, \
             tc.tile_pool(name="ps", bufs=3, space="PSUM") as ps:

            def load_chunks(pool, ap2d, nm, blocks=None):
                out = []
                for kc in range(MC):
                    t = pool.tile([P, ap2d.shape[1]], ap2d.dtype,
                                  tag=f"{nm}{kc}", name=f"{nm}{kc}")
                    if blocks is None:
                        nc.sync.dma_start(t[:], ap2d[kc * P:(kc + 1) * P, :])
                    out.append(t)
                return out

            def load_block(tiles, ap2d, off, blk):
                for kc in range(MC):
                    nc.sync.dma_start(
                        tiles[kc][:, off:off + blk],
                        ap2d[kc * P:(kc + 1) * P, off:off + blk])

            # ---- DMA in dependency-first order ----
            wq = load_chunks(cpool, wq_d, "wq")
            xqT = load_chunks(xpool, xqT_d, "xq")
            bq = cpool.tile_from(bq_d)
            wk = load_chunks(cpool, wk_d, "wk")
            xkT = load_chunks(xpool, xkT_d, "xk", blocks=[])
            load_block(xkT, xkT_d, *kblk[0])
            wv = load_chunks(cpool, wv_d, "wv")
            xvT = load_chunks(xpool, xvT_d, "xv", blocks=[])
            load_block(xvT, xvT_d, *kblk[0])
            bk = cpool.tile_from(bk_d)
            bv = cpool.tile_from(bv_d)
            mb = cpool.tile_from(mb_d)
            for off, blk in kblk[1:]:
                load_block(xkT, xkT_d, off, blk)
                load_block(xvT, xvT_d, off, blk)
            wo = load_chunks(cpool, wo_d, "wo")
            bo2 = cpool.tile_from(bo2_d)

            ones_w = cpool.tile([1, D], MM_DT)
            nc.vector.memset(ones_w[:], 1.0)
            bo_bc = cpool.tile([P, 2 * D], F32)
            nc.gpsimd.partition_broadcast(bo_bc[:], bo2[:])

            # ---- PE warmup (HAM un-throttle) + ACT exp-table preload ----
            wps = ps.tile([P, LQ], F32, tag="sc", name="wps")
            for i in range(24):
                nc.tensor.matmul(wps[:, 0:512], ones_w[0:1, 0:P],
                                 ones_w[0:1, :], start=True, stop=True)
            dxp = npool.tile([1, 1], F32, tag="dxp")
            nc.scalar.activation(dxp[:], wps[0:1, 0:1], AF.Exp)

            # ---- Q/K projections for dmodel chunk 0 (rest interleaved) ----
            qT = [prpool.tile([P, LQ], MM_DT, tag=f"qT{m}", name=f"qT{m}")
                  for m in range(MC)]
            kT = [prpool.tile([P, kvcap], MM_DT, tag=f"kT{m}", name=f"kT{m}")
                  for m in range(MC)]
            V = [prpool.tile([P, H * 65], MM_DT, tag=f"V{t}", name=f"V{t}")
                 for t in range(kvc)]
            xsT2 = [prpool.tile([P, LQ], MM_DT, tag=f"xs{hp}", name=f"xsT2_{hp}")
                    for hp in range(MC)]

            qpp = {}

            def q_unit(m, s, engine):
                if s == 0:
                    qpp[m] = ps.tile([P, LQ], F32, tag="sc", name=f"qpp{m}")
                pp = qpp[m]
                for kc in range(MC):
                    nc.tensor.matmul(
                        pp[:, s * 512:(s + 1) * 512],
                        wq[kc][:, m * P:(m + 1) * P],
                        xqT[kc][:, s * 512:(s + 1) * 512],
                        start=kc == 0, stop=kc == MC - 1)
                if s == 1:
                    if engine == "act":
                        nc.scalar.activation(qT[m][:], pp[:], AF.Identity,
                                             bias=bq[:, m:m + 1])
                    else:
                        nc.vector.tensor_scalar_add(qT[m][:], pp[:],
                                                    bq[:, m:m + 1])

            def k_unit(m, bi):
                off, blk = kblk[bi]
                pk = ps.tile([P, LQ], F32, tag="sc", name=f"pk{m}_{bi}")
                for kc in range(MC):
                    nc.tensor.matmul(
                        pk[:, 0:blk],
                        wk[kc][:, m * P:(m + 1) * P],
                        xkT[kc][:, off:off + blk],
                        start=kc == 0, stop=kc == MC - 1)
                nc.vector.tensor_scalar_add(kT[m][:, off:off + blk],
                                            pk[:, 0:blk], bk[:, m:m + 1])

            def v_unit(t):
                pv = ps.tile([P, LQ], F32, tag="sc", name=f"pv{t}")
                for kc in range(MC):
                    nc.tensor.matmul(pv[:, 0:D],
                                     xvT[kc][:, t * P:(t + 1) * P],
                                     wv[kc][:, :], start=kc == 0, stop=False)
                nc.tensor.matmul(pv[:, 0:D], ones_w[0:1, 0:P], bv[0:1, :],
                                 start=False, stop=True)
                vv = V[t].rearrange("p (g d) -> p g d", d=65)
                nc.vector.tensor_copy(vv[:, :, 0:64],
                                      pv[:, 0:D].rearrange("p (g d) -> p g d",
                                                           d=64))
                nc.vector.memset(vv[:, :, 64:65], 1.0)

            q_unit(0, 0, "act")
            q_unit(0, 1, "act")
            k_unit(0, 0)
            v_unit(0)

            # side work interleaved into attention PE slack: per head, a
            # list of per-chunk-step unit lists. Head 0 carries the V
            # projections (JIT, one chunk ahead of attnV) plus the two
            # remaining K blocks of dmodel-chunk 0 (due before scores c=4
            # and c=8). Q/K dmodel-chunk m is projected during head 2m-1.
            side = {h: [[] for _ in range(kvc)] for h in range(H)}
            for t in range(1, kvc):
                side[0][t - 1].append(lambda t=t: v_unit(t))
            for j, bi in enumerate(range(1, len(kblk))):
                step = min(2 + 3 * j, 4 * bi - 1, kvc - 1)
                side[0][step].append(lambda bi=bi: k_unit(0, bi))
            for m in range(1, MC):
                units = [lambda m=m: q_unit(m, 0, "dve"),
                         lambda m=m: q_unit(m, 1, "dve")]
                units += [lambda m=m, bi=bi: k_unit(m, bi)
                          for bi in range(len(kblk))]
                h = min(2 * m - 1, H - 1)
                for i, u in enumerate(units):
                    side[h][min(i, kvc - 1)].append(u)

            # ---- flash attention per head, chunk-pipelined ----
            def scores_chunk(h, c):
                hp, po = h // 2, 64 * (h % 2)
                ss = ps.tile([P, LQ], F32, tag="sc", name=f"ss_h{h}_{c}")
                for qh in range(2):
                    nc.tensor.matmul(
                        ss[:, qh * 512:(qh + 1) * 512],
                        kT[hp][po:po + 64, c * P:(c + 1) * P],
                        qT[hp][po:po + 64, qh * 512:(qh + 1) * 512],
                        start=True, stop=True)
                a = apool.tile([P, LQ], MM_DT, tag="at", name=f"at_h{h}_{c}")
                nc.scalar.activation(a[:], ss[:], AF.Exp,
                                     bias=mb[:, c:c + 1], scale=0.125)
                return a

            def attnv_chunk(h, c, xs, a):
                for qh in range(2):
                    nc.tensor.matmul(
                        xs[:, qh * 512:(qh + 1) * 512],
                        V[c][:, 65 * h:65 * h + 65],
                        a[:, qh * 512:(qh + 1) * 512],
                        start=c == 0, stop=c == kvc - 1)

            for h in range(H):
                hp, po = h // 2, 64 * (h % 2)
                xs = ps.tile([65, LQ], F32, tag="xs", bufs=1, name=f"xs_h{h}")
                at_prev = None
                for c in range(kvc):
                    a = scores_chunk(h, c)
                    for u in side[h][c]:
                        u()
                    if at_prev is not None:
                        attnv_chunk(h, c - 1, xs, at_prev)
                    at_prev = a
                attnv_chunk(h, kvc - 1, xs, at_prev)
                # normalize: row 64 holds the softmax denominator
                rec = npool.tile([1, LQ], F32, tag="rec")
                nc.vector.reciprocal_approx_fast(rec[:], xs[64:65, :])
                bc = npool.tile([64, LQ], F32, tag="bc")
                nc.gpsimd.partition_broadcast(bc[:], rec[:])
                nc.vector.tensor_mul(xsT2[hp][po:po + 64, :],
                                     xs[0:64, :], bc[:])

            # ---- output projection ----
            for q2 in range(QT // 2):
                po_ = ps.tile([P, LQ], F32, tag="sc", name=f"po{q2}")
                for sub in range(2):
                    qt = 2 * q2 + sub
                    for hp in range(MC):
                        nc.tensor.matmul(po_[:, sub * 512:(sub + 1) * 512],
                                         xsT2[hp][:, qt * P:(qt + 1) * P],
                                         wo[hp][:, :],
                                         start=hp == 0, stop=hp == MC - 1)
                osb = opool.tile([P, 2 * D], F32, tag="osb")
                nc.vector.tensor_add(osb[:], po_[:], bo_bc[:])
                for sub in range(2):
                    qt = 2 * q2 + sub
                    nc.sync.dma_start(out_d[qt * P:(qt + 1) * P, :],
                                      osb[:, sub * 512:(sub + 1) * 512])

    nc.compile()
    return nc


def _host_inputs(query, key, value, mask, Wq, bq, Wk, bk, Wv, bv, Wo, bo,
                 kvc=None):
    """Build the 8 per-core input maps (all rank-dependence lives here)."""
    f32 = np.float32
    if kvc is None:
        kvc = _plan(mask)
    kvcap = kvc * P
    wq_ = np.ascontiguousarray(Wq).astype(MM_NP)
    wk_ = np.ascontiguousarray(Wk).astype(MM_NP)
    wv_ = np.ascontiguousarray(Wv).astype(MM_NP)
    wo_ = np.ascontiguousarray(Wo).astype(MM_NP)
    bq_ = np.ascontiguousarray(bq.astype(f32).reshape(MC, P).T)
    bk_ = np.ascontiguousarray(bk.astype(f32).reshape(MC, P).T)
    bv_ = bv.astype(MM_NP).reshape(1, D)
    bo2_ = np.tile(bo.astype(f32), 2).reshape(1, 2 * D)
    in_maps = []
    per_batch = {}
    for b in range(B):
        idx = np.flatnonzero(np.asarray(mask[b]) != 0)
        n = len(idx)
        xk = np.zeros((kvcap, D), f32)
        xv = np.zeros((kvcap, D), f32)
        xk[:n] = np.asarray(key[b], f32)[idx]
        xv[:n] = np.asarray(value[b], f32)[idx]
        mbias = np.full(kvcap, MASK_BIAS, f32)
        mbias[:n] = 0.0
        per_batch[b] = (
            np.ascontiguousarray(xk.T).astype(MM_NP),
            np.ascontiguousarray(xv.T).astype(MM_NP),
            np.ascontiguousarray(mbias.reshape(kvc, P).T),
        )
    for c in range(N_CORES):
        b, half = c // 2, c % 2
        sl = slice(half * LQ, (half + 1) * LQ)
        xqT = np.ascontiguousarray(np.asarray(query[b], f32)[sl].T).astype(MM_NP)
        xkT_, xvT_, mb_ = per_batch[b]
        in_maps.append({
            "xqT": xqT, "xkT": xkT_, "xvT": xvT_,
            "wq": wq_, "wk": wk_, "wv": wv_, "wo": wo_,
            "bq": bq_, "bk": bk_, "bv": bv_, "bo2": bo2_, "mb": mb_,
        })
    return in_maps


def kernel(query, key, value, mask, Wq, bq, Wk, bk, Wv, bv, Wo, bo):
    kvc = _plan(mask)
    if kvc not in _cache:
        _cache[kvc] = _build(kvc)
    _cache["nc"] = _cache[kvc]
    nc = _cache[kvc]
    in_maps = _host_inputs(query, key, value, mask,
                           Wq, bq, Wk, bk, Wv, bv, Wo, bo, kvc=kvc)
    res = run_bass_kernel_spmd(nc, in_maps, list(range(N_CORES))).results
    out = np.empty((B, L, D), np.float32)
    for c in range(N_CORES):
        b, half = c // 2, c % 2
        out[b, half * LQ:(half + 1) * LQ, :] = res[c]["out"]
    return out


# revision 8
# speedup vs baseline: 1.1448x; 1.1448x over previous
"""Multi-head attention (B=4, L=2048, D=512, H=8) on 8 Trainium2 cores.

Sharding: core c handles batch b = c//2, query rows [(c%2)*1024, +1024).

Key trick: the key-mask zeroes ~half the KV positions and is known on the
host, so K/V are COMPACTED on the host to the unmasked positions (padded
to a multiple of 128; pad columns get a -1e30 score bias so exp()=0).
This halves scores/exp/attnV work. Each core projects the compacted K/V
for its whole batch itself (cheap), so no collective is needed.

Schedule: the scalar engine (72 exps of [128,1024]) and the PE are
co-critical; the layout keeps both near-saturated:
 - warmup matmuls + a dummy exp run during the DMA preamble (HAM warm,
   ACT exp tables resident before the first real chunk)
 - only Q/K dmodel-chunk 0 is projected up front; V projection runs JIT
   inside head 0, remaining Q/K chunks stream into later heads' PE slack
 - qT/kT are double-stored with partition halves swapped (cheap SBUF
   DMA), so even/odd chunk scores matmuls (K=64) land in different PE
   row groups and execute concurrently (row tiling)
 - attnV lags scores by 2 chunks so PE never waits on exp or the
   previous head's normalize

Device layouts (per core):
  xqT (512, 1024), xkT/xvT (512, KVCAP)  inputs, dmodel on partitions
  qT (128, 1024) x4 / kT (128, KVCAP) x4 projections kept transposed:
      head h lives in dmodel-chunk tile h//2 at partition offset 64*(h%2)
  qTs/kTs: same with partition halves swapped (for row-group pairing)
  V (128, 520) x KVC   V natural layout per kv chunk; head h at cols
      [65h, 65h+64), col 65h+64 = ones (softmax denominator)
  scores (128kv, 1024q) PSUM; exp+mask+scale fused into one ACT op
  xs (65, 1024) PSUM, row 64 = softmax denominator
"""
import numpy as np
import ml_dtypes

import concourse.bacc as bacc
import concourse.bass as bass
import concourse.mybir as mybir
import concourse.tile as tile
from concourse.bass_utils import run_bass_kernel_spmd

F32 = mybir.dt.float32
BF16 = mybir.dt.bfloat16
AF = mybir.ActivationFunctionType

B, L, D = 4, 2048, 512
H, DK = 8, 64
N_CORES = 8
LQ = L // 2            # query rows per core
P = 128
QT = LQ // P           # 8 query tiles of 128
MC = D // P            # 4 dmodel chunks
MASK_BIAS = np.float32(-1e30)

MM_DT = BF16
MM_NP = ml_dtypes.bfloat16

_cache = {}


def _plan(mask):
    """KV chunk count after host-side compaction (multiple-of-128 pad)."""
    counts = np.asarray(mask).astype(bool).sum(axis=1)
    kvc = int(np.ceil((counts.max() + 1e-9) / P))
    return max(kvc, 2)


def _build(kvc):
    kvcap = kvc * P
    nc = bacc.Bacc("TRN2", target_bir_lowering=False, debug=False,
                   num_devices=N_CORES)

    xqT_d = nc.dram_tensor("xqT", [D, LQ], MM_DT, kind="ExternalInput").ap()
    xkT_d = nc.dram_tensor("xkT", [D, kvcap], MM_DT, kind="ExternalInput").ap()
    xvT_d = nc.dram_tensor("xvT", [D, kvcap], MM_DT, kind="ExternalInput").ap()
    wq_d = nc.dram_tensor("wq", [D, D], MM_DT, kind="ExternalInput").ap()
    wk_d = nc.dram_tensor("wk", [D, D], MM_DT, kind="ExternalInput").ap()
    wv_d = nc.dram_tensor("wv", [D, D], MM_DT, kind="ExternalInput").ap()
    wo_d = nc.dram_tensor("wo", [D, D], MM_DT, kind="ExternalInput").ap()
    bq_d = nc.dram_tensor("bq", [P, MC], F32, kind="ExternalInput").ap()
    bk_d = nc.dram_tensor("bk", [P, MC], F32, kind="ExternalInput").ap()
    bv_d = nc.dram_tensor("bv", [1, D], F32, kind="ExternalInput").ap()
    bo2_d = nc.dram_tensor("bo2", [1, 2 * D], F32, kind="ExternalInput").ap()
    mb_d = nc.dram_tensor("mb", [P, kvc], F32, kind="ExternalInput").ap()
    out_d = nc.dram_tensor("out", [LQ, D], F32, kind="ExternalOutput").ap()

    # 512-column blocks of the compacted KV extent
    kblk = []
    off = 0
    while off < kvcap:
        blk = min(512, kvcap - off)
        kblk.append((off, blk))
        off += blk
    NB = len(kblk)

    with tile.TileContext(nc) as tc:
        with tc.tile_pool(name="const", bufs=1) as cpool, \
             tc.tile_pool(name="xin", bufs=1) as xpool, \
             tc.tile_pool(name="proj", bufs=1) as prpool, \
             tc.tile_pool(name="attn", bufs=6) as apool, \
             tc.tile_pool(name="norm", bufs=2) as npool, \
             tc.tile_pool(name="outp", bufs=2) as opool, \
             tc.tile_pool(name="ps", bufs=3, space="PSUM") as ps:

            def alloc_chunks(pool, ap2d, nm):
                return [pool.tile([P, ap2d.shape[1]], ap2d.dtype,
                                  tag=f"{nm}{kc}", name=f"{nm}{kc}")
                        for kc in range(MC)]

            def load_all(tiles, ap2d):
                for kc in range(MC):
                    nc.sync.dma_start(tiles[kc][:], ap2d[kc * P:(kc + 1) * P, :])

            def load_block(tiles, ap2d, off, blk):
                for kc in range(MC):
                    nc.sync.dma_start(
                        tiles[kc][:, off:off + blk],
                        ap2d[kc * P:(kc + 1) * P, off:off + blk])

            # ---- DMA in dependency-first order ----
            wq = alloc_chunks(cpool, wq_d, "wq")
            load_all(wq, wq_d)
            xqT = alloc_chunks(xpool, xqT_d, "xq")
            load_all(xqT, xqT_d)
            bq = cpool.tile_from(bq_d)
            wk = alloc_chunks(cpool, wk_d, "wk")
            load_all(wk, wk_d)
            xkT = alloc_chunks(xpool, xkT_d, "xk")
            load_block(xkT, xkT_d, *kblk[0])
            wv = alloc_chunks(cpool, wv_d, "wv")
            load_all(wv, wv_d)
            xvT = alloc_chunks(xpool, xvT_d, "xv")
            load_block(xvT, xvT_d, *kblk[0])
            bk = cpool.tile_from(bk_d)
            bv = cpool.tile_from(bv_d)
            mb = cpool.tile_from(mb_d)
            for off, blk in kblk[1:]:
                load_block(xkT, xkT_d, off, blk)
                load_block(xvT, xvT_d, off, blk)
            wo = alloc_chunks(cpool, wo_d, "wo")
            load_all(wo, wo_d)
            bo2 = cpool.tile_from(bo2_d)

            ones_w = cpool.tile([1, D], MM_DT)
            nc.vector.memset(ones_w[:], 1.0)
            bo_bc = cpool.tile([P, 2 * D], F32)
            nc.gpsimd.partition_broadcast(bo_bc[:], bo2[:])
            bv_bc = cpool.tile([P, D], F32)
            nc.gpsimd.partition_broadcast(bv_bc[:], bv[:])

            # ---- PE warmup (HAM un-throttle) + ACT exp-table preload ----
            wps = ps.tile([P, LQ], F32, tag="sc", name="wps")
            for i in range(24):
                nc.tensor.matmul(wps[:, 0:512], ones_w[0:1, 0:P],
                                 ones_w[0:1, :], start=True, stop=True)
            dxp = npool.tile([1, 1], F32, tag="dxp")
            nc.scalar.activation(dxp[:], wps[0:1, 0:1], AF.Exp)

            # ---- persistent SBUF tiles ----
            qT = [prpool.tile([P, LQ], MM_DT, tag=f"qT{m}", name=f"qT{m}")
                  for m in range(MC)]
            qTs = [prpool.tile([P, LQ], MM_DT, tag=f"qTs{m}", name=f"qTs{m}")
                   for m in range(MC)]
            kT = [prpool.tile([P, kvcap], MM_DT, tag=f"kT{m}", name=f"kT{m}")
                  for m in range(MC)]
            kTs = [prpool.tile([P, kvcap], MM_DT, tag=f"kTs{m}", name=f"kTs{m}")
                   for m in range(MC)]
            V = [prpool.tile([P, H * 65], MM_DT, tag=f"V{t}", name=f"V{t}")
                 for t in range(kvc)]
            xsT2 = [prpool.tile([P, LQ], MM_DT, tag=f"xs{hp}", name=f"xsT2_{hp}")
                    for hp in range(MC)]

            def swap_copy(dst, src, off, width):
                nc.sync.dma_start(dst[0:64, off:off + width],
                                  src[64:128, off:off + width])
                nc.sync.dma_start(dst[64:128, off:off + width],
                                  src[0:64, off:off + width])

            def q_unit(m, engine):
                pp = ps.tile([P, LQ], F32, tag="sc", name=f"qpp{m}")
                for kc in range(MC):
                    for s in range(2):
                        nc.tensor.matmul(
                            pp[:, s * 512:(s + 1) * 512],
                            wq[kc][:, m * P:(m + 1) * P],
                            xqT[kc][:, s * 512:(s + 1) * 512],
                            start=kc == 0, stop=kc == MC - 1)
                if engine == "act":
                    nc.scalar.activation(qT[m][:], pp[:], AF.Identity,
                                         bias=bq[:, m:m + 1])
                else:
                    nc.vector.tensor_scalar_add(qT[m][:], pp[:], bq[:, m:m + 1])
                swap_copy(qTs[m], qT[m], 0, LQ)

            def k_unit(m, bi):
                off, blk = kblk[bi]
                pk = ps.tile([P, LQ], F32, tag="sc", name=f"pk{m}_{bi}")
                for kc in range(MC):
                    nc.tensor.matmul(
                        pk[:, 0:blk],
                        wk[kc][:, m * P:(m + 1) * P],
                        xkT[kc][:, off:off + blk],
                        start=kc == 0, stop=kc == MC - 1)
                nc.vector.tensor_scalar_add(kT[m][:, off:off + blk],
                                            pk[:, 0:blk], bk[:, m:m + 1])
                swap_copy(kTs[m], kT[m], off, blk)

            def v_unit(t):
                pv = ps.tile([P, LQ], F32, tag="sc", name=f"pv{t}")
                for kc in range(MC):
                    nc.tensor.matmul(pv[:, 0:D],
                                     xvT[kc][:, t * P:(t + 1) * P],
                                     wv[kc][:, :],
                                     start=kc == 0, stop=kc == MC - 1)
                vv = V[t].rearrange("p (g d) -> p g d", d=65)
                nc.vector.tensor_add(
                    vv[:, :, 0:64],
                    pv[:, 0:D].rearrange("p (g d) -> p g d", d=64),
                    bv_bc.rearrange("p (g d) -> p g d", d=64))
                nc.vector.memset(vv[:, :, 64:65], 1.0)

            q_unit(0, "act")
            k_unit(0, 0)
            v_unit(0)
            v_unit(1)

            # ---- flash attention ----
            def scores_mms(h, c, qh, ss):
                hp, po = h // 2, 64 * (h % 2)
                if c % 2 == 1 and h > 0:
                    src_k, src_q, o = kTs[hp], qTs[hp], 64 - po
                else:
                    src_k, src_q, o = kT[hp], qT[hp], po
                nc.tensor.matmul(
                    ss[:, qh * 512:(qh + 1) * 512],
                    src_k[o:o + 64, c * P:(c + 1) * P],
                    src_q[o:o + 64, qh * 512:(qh + 1) * 512],
                    start=True, stop=True)

            def exp_chunk(h, c, ss):
                a = apool.tile([P, LQ], MM_DT, tag="at", name=f"at_h{h}_{c}")
                nc.scalar.activation(a[:], ss[:], AF.Exp,
                                     bias=mb[:, c:c + 1], scale=0.125)
                return a

            def attnv_chunk(h, c, xs, a):
                for qh in range(2):
                    nc.tensor.matmul(
                        xs[:, qh * 512:(qh + 1) * 512],
                        V[c][:, 65 * h:65 * h + 65],
                        a[:, qh * 512:(qh + 1) * 512],
                        start=c == 0, stop=c == kvc - 1)

            def normalize(h, xs):
                hp, po = h // 2, 64 * (h % 2)
                rec = npool.tile([1, LQ], F32, tag="rec")
                nc.vector.reciprocal_approx_fast(rec[:], xs[64:65, :])
                bc = npool.tile([64, LQ], F32, tag="bc")
                nc.gpsimd.partition_broadcast(bc[:], rec[:])
                nc.vector.tensor_mul(xsT2[hp][po:po + 64, :],
                                     xs[0:64, :], bc[:])

            # side work: head 0 carries V JIT + remaining K blocks of m=0;
            # dmodel chunk m is projected during heads 2m-2 / 2m-1.
            side0 = [[] for _ in range(kvc)]
            for t in range(2, kvc):
                side0[t - 2].append(lambda t=t: v_unit(t))
            for j, bi in enumerate(range(1, NB)):
                step = min(2 + 3 * j, 4 * bi - 2, kvc - 1)
                side0[step].append(lambda bi=bi: k_unit(0, bi))

            sidep = {h: [] for h in range(1, H)}  # per pair-step lists
            for m in range(1, MC):
                units = [lambda m=m: q_unit(m, "dve")]
                units += [lambda m=m, bi=bi: k_unit(m, bi) for bi in range(NB)]
                ha, hb = max(2 * m - 2, 1), 2 * m - 1
                for i, u in enumerate(units):
                    sidep[ha if (i < 2 and ha != hb) else hb].append(u)
            for h in range(1, H):
                nsteps = (kvc + 1) // 2
                lst = sidep[h]
                sidep[h] = [[] for _ in range(nsteps)]
                for i, u in enumerate(lst):
                    sidep[h][min(i, nsteps - 1)].append(u)

            # head 0: unpaired chunk loop with V JIT
            xs = ps.tile([65, LQ], F32, tag="xs", bufs=1, name="xs_h0")
            ats = {}
            for c in range(kvc):
                ss = ps.tile([P, LQ], F32, tag="sc", name=f"ss_h0_{c}")
                for qh in range(2):
                    scores_mms(0, c, qh, ss)
                ats[c] = exp_chunk(0, c, ss)
                for u in side0[c]:
                    u()
                if c >= 2:
                    attnv_chunk(0, c - 2, xs, ats.pop(c - 2))
            for c in (kvc - 2, kvc - 1):
                attnv_chunk(0, c, xs, ats.pop(c))
            normalize(0, xs)

            # heads 1-7: paired even/odd chunks in alternating row groups
            for h in range(1, H):
                xs = ps.tile([65, LQ], F32, tag="xs", bufs=1, name=f"xs_h{h}")
                ats = {}
                done = 0
                for step, pc in enumerate(range(0, kvc, 2)):
                    cs = [pc] + ([pc + 1] if pc + 1 < kvc else [])
                    sst = {c: ps.tile([P, LQ], F32, tag="sc",
                                      name=f"ss_h{h}_{c}") for c in cs}
                    for qh in range(2):
                        for c in cs:
                            scores_mms(h, c, qh, sst[c])
                    for c in cs:
                        ats[c] = exp_chunk(h, c, sst[c])
                    for u in sidep[h][step]:
                        u()
                    while done <= pc - 2:
                        attnv_chunk(h, done, xs, ats.pop(done))
                        done += 1
                while done < kvc:
                    attnv_chunk(h, done, xs, ats.pop(done))
                    done += 1
                normalize(h, xs)

            # ---- output projection ----
            for q2 in range(QT // 2):
                po_ = ps.tile([P, LQ], F32, tag="sc", name=f"po{q2}")
                for sub in range(2):
                    qt = 2 * q2 + sub
                    for hp in range(MC):
                        nc.tensor.matmul(po_[:, sub * 512:(sub + 1) * 512],
                                         xsT2[hp][:, qt * P:(qt + 1) * P],
                                         wo[hp][:, :],
                                         start=hp == 0, stop=hp == MC - 1)
                osb = opool.tile([P, 2 * D], F32, tag="osb")
                nc.vector.tensor_add(osb[:], po_[:], bo_bc[:])
                for sub in range(2):
                    qt = 2 * q2 + sub
                    nc.sync.dma_start(out_d[qt * P:(qt + 1) * P, :],
                                      osb[:, sub * 512:(sub + 1) * 512])

    nc.compile()
    return nc


def _host_inputs(query, key, value, mask, Wq, bq, Wk, bk, Wv, bv, Wo, bo,
                 kvc=None):
    """Build the 8 per-core input maps (all rank-dependence lives here)."""
    f32 = np.float32
    if kvc is None:
        kvc = _plan(mask)
    kvcap = kvc * P
    wq_ = np.ascontiguousarray(Wq).astype(MM_NP)
    wk_ = np.ascontiguousarray(Wk).astype(MM_NP)
    wv_ = np.ascontiguousarray(Wv).astype(MM_NP)
    wo_ = np.ascontiguousarray(Wo).astype(MM_NP)
    bq_ = np.ascontiguousarray(bq.astype(f32).reshape(MC, P).T)
    bk_ = np.ascontiguousarray(bk.astype(f32).reshape(MC, P).T)
    bv_ = bv.astype(f32).reshape(1, D)
    bo2_ = np.tile(bo.astype(f32), 2).reshape(1, 2 * D)
    in_maps = []
    per_batch = {}
    for b in range(B):
        idx = np.flatnonzero(np.asarray(mask[b]) != 0)
        n = len(idx)
        xk = np.zeros((kvcap, D), f32)
        xv = np.zeros((kvcap, D), f32)
        xk[:n] = np.asarray(key[b], f32)[idx]
        xv[:n] = np.asarray(value[b], f32)[idx]
        mbias = np.full(kvcap, MASK_BIAS, f32)
        mbias[:n] = 0.0
        per_batch[b] = (
            np.ascontiguousarray(xk.T).astype(MM_NP),
            np.ascontiguousarray(xv.T).astype(MM_NP),
            np.ascontiguousarray(mbias.reshape(kvc, P).T),
        )
    for c in range(N_CORES):
        b, half = c // 2, c % 2
        sl = slice(half * LQ, (half + 1) * LQ)
        xqT = np.ascontiguousarray(np.asarray(query[b], f32)[sl].T).astype(MM_NP)
        xkT_, xvT_, mb_ = per_batch[b]
        in_maps.append({
            "xqT": xqT, "xkT": xkT_, "xvT": xvT_,
            "wq": wq_, "wk": wk_, "wv": wv_, "wo": wo_,
            "bq": bq_, "bk": bk_, "bv": bv_, "bo2": bo2_, "mb": mb_,
        })
    return in_maps


def kernel(query, key, value, mask, Wq, bq, Wk, bk, Wv, bv, Wo, bo):
    kvc = _plan(mask)
    if kvc not in _cache:
        _cache[kvc] = _build(kvc)
    _cache["nc"] = _cache[kvc]
    nc = _cache[kvc]
    in_maps = _host_inputs(query, key, value, mask,
                           Wq, bq, Wk, bk, Wv, bv, Wo, bo, kvc=kvc)
    res = run_bass_kernel_spmd(nc, in_maps, list(range(N_CORES))).results
    out = np.empty((B, L, D), np.float32)
    for c in range(N_CORES):
        b, half = c // 2, c % 2
        out[b, half * LQ:(half + 1) * LQ, :] = res[c]["out"]
    return out
